# revision 40
# baseline (speedup 1.0000x reference)
"""Bezier-to-image Gaussian splat kernel for Trainium2 (8 NeuronCores).

Reference computation (per sample b of 256):
    T = warped cubic Bernstein basis (30, 4)
    points = einsum('nk,blkc->blnc', T, x.reshape(B,160,4,2))   # (B,160,30,2)
    gx[b,l,i,n] = exp(-(i/60 - X[b,l,n])^2 / 2e-4)
    out[b,i,j]  = min(sum_{l,n} gx[b,l,i,n]*gy[b,l,j,n], 1)     # (B,60,60)

Strategy: pure data parallel, 32 samples per core.

Input path: the naive "partition = ctrl-point-index" DMA degenerates into
20k 8-byte packets (~105us aggregate across every DMA queue -- the packet
rate, not bandwidth, is the limit).  Instead x is DMA'd CONTIGUOUSLY as
[32, 1280] (~2us), transposed on the Tensor engine in ten [32,128] tiles
into T_all[(dl,k,co), (j,b)], and the per-sample point coordinates
r = 60*X come from eight [128,128] stationary matmuls St_{a,co}^T @ T_col
that fold the Bernstein basis, the curve regrouping (chunk c = a*10+j
holds curves l = 16j+4a+lg), and the dead-row hole (rows n in {30,31} of
each 32-strip contract ALL EIGHT values of the curve with -120, so
r_dead <= -60 with probability 1 - 1e-10 and those Gaussians vanish).

Compute per sample (Scalar-engine exp at 1 elem/cycle = 4.3us is the
floor; DVE's broadcast subtract at 1x mode, 5.2us, is the pace):
  d[p,i] = i - r  via ONE batched DVE tensor_tensor (iota fp16 const vs
  r broadcast straight out of PSUM), Derivative_Erf LUT on the Scalar
  engine (= 2/sqrt(pi)*exp(-x^2)) in one [128,4800] pass, image
  accumulation sum_c GxT_c^T @ GyT_c on the Tensor engine, PSUM drained
  per sample on the Scalar engine + DMA out (min+scale on the host over
  the gathered output -- keeping the DVE and the img chain decoupled).
"""

import math

import numpy as np
import orjson

import bass_rust
import concourse.bass as bass
import concourse.mybir as mybir
import concourse.tile as tile
from concourse.bass_utils import run_bass_kernel_spmd

B, L, N, W = 256, 160, 30, 60
NCORES = 8
BC = B // NCORES          # samples per core
ALPHA = 2e-4
KEXP = 1.0 / (W * W * ALPHA)          # exponent scale in cell units: 1/0.72
SDERF = math.sqrt(KEXP)               # Derivative_Erf input scale
DERF_FIX = math.pi / 4.0              # undo (2/sqrt(pi))^2 from Derivative_Erf
CHUNKS = 40                           # chunk c = a*10 + j: curves 16j+4a+lg
PTS = 128                             # partition p = 32*lg + n
CW = 60                               # cells per axis
NT = 10                               # transpose tiles (1280 = 10*128)
R_DEAD = -120.0                       # dead-row contraction weight

LAST_RESULTS = None  # test harness reads profiling info from here


def _basis_T() -> np.ndarray:
    t = np.arange(N, dtype=np.float32) / np.float32(N)
    t = 2 * t**3 - 3 * t**2 + 2 * t
    t_3_0 = t**3
    t_2_1 = t**2 - t_3_0
    t_1_2 = t_3_0 - 2 * t**2 + t
    t_0_3 = (1 - t) ** 3
    return np.stack([t_3_0, 3 * t_2_1, 3 * t_1_2, t_0_3], axis=1).astype(np.float32)


def _build_stationaries() -> np.ndarray:
    """St_all[(dl,k,co), (s, 32*lg+n)] with s = co*4 + a: the r-matmul
    weights.  St^T @ T_col gives r[32*lg+n, (co,a,j)] = 60*coord of sample
    n of curve 16j+4a+lg (j selected by the rhs column)."""
    basis = W * _basis_T()                        # (30, 4): 60*T[n,k]
    st = np.zeros((128, 8 * 128), dtype=np.float32)
    for co in range(2):
        for a in range(4):
            s = co * 4 + a
            for lg in range(4):
                dl = 4 * a + lg
                for k in range(4):
                    for n in range(N):
                        st[dl * 8 + k * 2 + co, s * 128 + 32 * lg + n] = basis[n, k]
                # dead rows: contract all 8 curve values with -120
                for n in (30, 31):
                    st[dl * 8 : dl * 8 + 8, s * 128 + 32 * lg + n] = R_DEAD
    return st


def _legalize_waits(nc, max_waits: int = 1):
    """Walrus rejects engine instructions carrying more than ~1 sync wait
    ("Too many sync wait commands").  Hoist excess waits onto same-engine
    Drain instructions inserted immediately before the offender."""
    js = orjson.loads(mybir.module_to_json_bytes(nc.m))
    ctr = 0
    for f in js["functions"]:
        for bb in f["blocks"]:
            out = []
            changed = False
            for inst in bb["instructions"]:
                si = inst.get("sync_info")
                waits = si.get("on_wait") if si else None
                if waits and len(waits) > max_waits:
                    keep = waits[:max_waits]
                    for w in waits[max_waits:]:
                        ctr += 1
                        out.append({
                            "debug": inst.get("debug", 0),
                            "engine": inst["engine"],
                            "ins": [], "outs": [],
                            "name": f"waitfix-{ctr}",
                            "opcode": "Drain",
                            "sync_info": {"on_update": [], "on_wait": [w]},
                        })
                    si["on_wait"] = keep
                    changed = True
                out.append(inst)
            if changed:
                bb["instructions"] = out
    if ctr:
        nc.m = bass_rust.module_from_json_bytes(orjson.dumps(js))
    return ctr


def build_program(legalize: bool = True):
    f32 = mybir.dt.float32
    f16 = mybir.dt.float16

    nc = bass.Bass("TRN2", target_bir_lowering=False, debug=False)

    x_t = nc.dram_tensor("x", [BC, L, 8], f32, kind="ExternalInput")
    y_t = nc.dram_tensor("y", [BC, W, W], f32, kind="ExternalOutput")

    st_d = nc.inline_tensor(_build_stationaries(), name="stAll")
    iota_np = np.tile(np.arange(CW, dtype=np.float16), (PTS, 1))  # (128, 60)
    iota_d = nc.inline_tensor(iota_np, name="iota60")
    ident_np = np.eye(32, dtype=np.float32)
    ident_d = nc.inline_tensor(ident_np, name="ident32")

    with tile.TileContext(nc) as tc, tc.tile_pool(name="const", bufs=1) as cpool, \
            tc.tile_pool(name="ddp", bufs=7) as dd_pool, \
            tc.tile_pool(name="ggp", bufs=11) as gg_pool, \
            tc.tile_pool(name="outp", bufs=12) as out_pool, \
            tc.tile_pool(name="tpsum", bufs=2, space="PSUM") as tps_pool, \
            tc.tile_pool(name="rpsum", bufs=3, space="PSUM") as rps_pool, \
            tc.tile_pool(name="imgpsum", bufs=3, space="PSUM") as img_pool:

        # Prologue constants (excess DMA-queue waits on PE consumers are
        # hoisted onto Drains by _legalize_waits)
        # x first on qSP (it gates the transposes); small consts on qAct
        xsb = cpool.tile([BC, L * 8], f32, tag="xsb")
        nc.sync.dma_start(xsb[:], x_t.ap().rearrange("b l k -> b (l k)"))
        st = cpool.tile([PTS, 8 * PTS], f32, tag="st")
        nc.sync.dma_start(st[:], st_d.ap())
        ident = cpool.tile([32, 32], f32, tag="ident")
        nc.scalar.dma_start(ident[:], ident_d.ap())
        iot = cpool.tile([PTS, CW], f16, tag="iota")
        nc.scalar.dma_start(iot[:], iota_d.ap())

        # PE transpose into T_all[(dl,k,co), (j, b)]
        t_all = cpool.tile([PTS, NT * BC], f32, tag="tall")
        for j in range(NT):
            tp = tps_pool.tile([PTS, BC], f32, tag="tp")
            nc.tensor.transpose(tp[:], xsb[:, PTS * j : PTS * (j + 1)], ident[:])
            # alternate drain engines so the copy chain pipelines 2-wide
            if j % 2 == 0:
                nc.vector.tensor_copy(t_all[:, BC * j : BC * (j + 1)], tp[:])
            else:
                nc.scalar.copy(t_all[:, BC * j : BC * (j + 1)], tp[:])
        # b-outer view for the block-r matmuls (stream order (b, j))
        t_bj = t_all[:].rearrange("p (j b) -> p b j", b=BC)

        # r for a BLOCK of 4 samples per stationary load: one matmul per
        # (block, s) streaming 40 columns, so each [128,128] LDWEIGHTS is
        # amortized over 4 samples.  Sample slots are padded to 128 f32
        # (512B) so a block is exactly one PSUM bank -- matmul outputs
        # must not straddle banks.
        RBLK = 4

        def emit_rblock_mm(blk, s, rt):
            out_v = rt[:].rearrange("p (b q) -> p b q", q=PTS)[:, :, 10 * s : 10 * s + 10]
            nc.tensor.matmul(
                out_v,
                lhsT=st[:, PTS * s : PTS * (s + 1)],
                rhs=t_bj[:, RBLK * blk : RBLK * (blk + 1), :],
                start=True,
                stop=True,
            )

        r_blocks = {}
        r_blocks[0] = rps_pool.tile([PTS, RBLK * PTS], f32, tag="rblk", name="rblk0")
        for s in range(8):
            emit_rblock_mm(0, s, r_blocks[0])
        pend = []

        def flush_one():
            # drain a finished image (two samples behind) out of PSUM on the
            # SCALAR engine, keeping the saturated DVE out of the
            # ACT->img->drain chain entirely (any DVE op here re-couples the
            # in-order DVE queue to the downstream and collapses the
            # pipeline).  min(scale*img, 1) happens on the host.
            img_o, b_o = pend.pop(0)
            osb = out_pool.tile([W, W], f32, tag="osb")
            nc.scalar.copy(osb[:], img_o[:])
            nc.sync.dma_start(y_t.ap()[b_o : b_o + 1], osb[:])

        for b in range(BC):
            blk, bl = b // RBLK, b % RBLK
            if len(pend) > 1:
                flush_one()
            # ---- banded distance + Gaussian, fp16; r read from PSUM ----
            dd = dd_pool.tile([PTS, 2 * CHUNKS * CW], f16, tag="dd")
            nc.vector.tensor_tensor(
                dd[:].rearrange("p (cs w) -> p cs w", w=CW),
                iot[:].rearrange("p (o w) -> p o w", o=1).broadcast_to(
                    [PTS, 2 * CHUNKS, CW]
                ),
                r_blocks[blk][:, PTS * bl : PTS * bl + 2 * CHUNKS]
                .rearrange("p (cs o) -> p cs o", o=1)
                .broadcast_to([PTS, 2 * CHUNKS, CW]),
                mybir.AluOpType.subtract,
            )
            gg = gg_pool.tile([PTS, 2 * CHUNKS * CW], f16, tag="gg")
            nc.scalar.activation(
                gg[:], dd[:],
                mybir.ActivationFunctionType.Derivative_Erf,
                bias=0.0, scale=SDERF,
            )

            # ---- image accumulation: sum_c GxT_c^T @ GyT_c ----
            # gg layout: [p, (co, c, w)] -> x plane [0:2400), y plane [2400:)
            img = img_pool.tile([W, W], f32, tag="img")
            for c in range(CHUNKS):
                nc.tensor.matmul(
                    img[:],
                    lhsT=gg[:, CW * c : CW * c + W],
                    rhs=gg[:, CW * (CHUNKS + c) : CW * (CHUNKS + c) + W],
                    start=(c == 0),
                    stop=(c == CHUNKS - 1),
                )

            # spread the NEXT block's r matmuls over this block's
            # samples (two stationaries after each image chain)
            if blk + 1 < BC // RBLK:
                if bl == 0:
                    r_blocks[blk + 1] = rps_pool.tile(
                        [PTS, RBLK * PTS], f32, tag="rblk",
                        name=f"rblk{blk + 1}",
                    )
                for s in (2 * bl, 2 * bl + 1):
                    emit_rblock_mm(blk + 1, s, r_blocks[blk + 1])

            pend.append((img, b))

        while pend:
            flush_one()

    if legalize:
        _legalize_waits(nc)
    return nc


_PROGRAM = None


def kernel(x: np.ndarray, _trace: bool = False) -> np.ndarray:
    global _PROGRAM, LAST_RESULTS
    assert x.shape == (B, L, 8) and x.dtype == np.float32, (x.shape, x.dtype)
    if _PROGRAM is None:
        _PROGRAM = build_program()
    nc = _PROGRAM
    shards = np.split(np.ascontiguousarray(x), NCORES, axis=0)
    in_maps = [{"x": s} for s in shards]
    res = run_bass_kernel_spmd(nc, in_maps, list(range(NCORES)), trace=_trace)
    LAST_RESULTS = res
    raw = np.concatenate([res.results[i]["y"] for i in range(NCORES)], axis=0)
    return np.minimum(raw * np.float32(DERF_FIX), np.float32(1.0)).astype(np.float32)
